# revision 14
# baseline (speedup 1.0000x reference)
"""MatAnyone memory-readout kernel for 8 Trainium2 NeuronCores.

Math (per batch b):
  sim[t,n]  = (-a_sq + two_ab - b_sq)[t,n] * ms[t] / sqrt(CK)
  aff       = softmax_t(sim)
  R[c,n]    = sum_t mv[c,t] * aff[t,n]
  out[c,n]  = R[c,n] * p[n] + lv[c,n] * (1 - p[n])

Sharding: 8 cores = 2 batches x 4 query-pixel shards (n = HW/4 = 576 each).
Within a core, two sequential passes over n-halves of 288 so all PSUM
accumulators (R0, R1, Z) fit in banks; softmax runs with t on partitions:
  sim matmul:  lhsT = [mk^2 ; mk] (K=128=2*CK), rhs = [-qe/8 ; qe*qk/4]
  psum       -= b_sq/8 (DVE, broadcast tile)
  E           = Exp(psum * ms_t)      (ACT, per-partition scale)
  R, Z        = matmuls contracting t, accumulated across all 144 t-tiles
  out         = R * (p/Z) + lv * (1-p)
Softmax max-subtraction is skipped: sim <= 0 always (negative weighted L2
distance), and max_t sim ~ 0, so exp never overflows and Z >= exp(max) is
well-scaled.
"""

import sys

for _p in ("/opt/trn_rl_repo", "/root/.axon_site/_ro/trn_rl_repo"):
    if _p not in sys.path:
        sys.path.insert(0, _p)

from contextlib import ExitStack

import numpy as np

import concourse.bass as bass
from concourse import mybir
from concourse.bacc import Bacc
from concourse.tile import TileContext
from concourse.bass_utils import run_bass_kernel_spmd

F32 = mybir.dt.float32
F32R = mybir.dt.float32r
EXP = mybir.ActivationFunctionType.Exp

B, CK, CV, T, H, W = 2, 64, 256, 8, 48, 48
HW = H * W            # 2304
THW = T * HW          # 18432
NCORE = HW // 4       # 576 query pixels per core
NH = NCORE // 2       # 288 per pass
TT = THW // 128       # 144 t-tiles
NPAIR = TT // 2       # 72 pairs of t-tiles
MKCH = 8              # t-tiles per streamed M2 chunk
SKEW = 3              # software-pipeline skew, in pairs

_CACHE = {}


def _f32r(ap):
    return ap.bitcast(F32R)


def build_program():
    nc = Bacc(name="matanyone_knn")

    cz_h = nc.declare_dram_parameter("c_onesz", [128, 2], F32R, isOutput=False)
    cb_h = nc.declare_dram_parameter("c_onesb", [1, 128], F32R, isOutput=False)
    ce_h = nc.declare_dram_parameter("c_eighth", [CK, 128], F32R, isOutput=False)
    qk_h = nc.declare_dram_parameter("qk", [CK, NCORE], F32, isOutput=False)
    qe_h = nc.declare_dram_parameter("qe", [CK, NCORE], F32, isOutput=False)
    mk_h = nc.declare_dram_parameter("mk", [CK, THW], F32R, isOutput=False)
    ms_h = nc.declare_dram_parameter("msT", [128, TT], F32, isOutput=False)
    mv_h = nc.declare_dram_parameter("mvT", [THW, CV], F32R, isOutput=False)
    lv_h = nc.declare_dram_parameter("lv", [CV, NCORE], F32, isOutput=False)
    p_h = nc.declare_dram_parameter("p", [1, NCORE], F32, isOutput=False)
    out_h = nc.declare_dram_parameter("out", [CV, NCORE], F32, isOutput=True)

    with TileContext(nc) as tc, ExitStack() as ctx:
        persist = ctx.enter_context(tc.tile_pool(name="persist", bufs=1))
        mvpool = ctx.enter_context(tc.tile_pool(name="mv", bufs=1))
        m2pool = ctx.enter_context(tc.tile_pool(name="m2", bufs=2))
        epool = ctx.enter_context(tc.tile_pool(name="E", bufs=2 * (SKEW + 2)))
        opool = ctx.enter_context(tc.tile_pool(name="O", bufs=2))
        smpool = ctx.enter_context(tc.tile_pool(name="small", bufs=2))
        ps_pair = ctx.enter_context(tc.tile_pool(name="pspair", bufs=2, space="PSUM"))
        ps_acc = ctx.enter_context(tc.tile_pool(name="psacc", bufs=1, space="PSUM"))

        # ---- constants / setup -------------------------------------------
        ones_z = persist.tile([128, 2], F32R, tag="ones_z")      # Z matmul lhsT
        nc.sync.dma_start(out=ones_z[:], in_=cz_h[:])
        ones_b = persist.tile([1, 128], F32R, tag="ones_b")      # K=1 broadcast lhsT
        nc.sync.dma_start(out=ones_b[:], in_=cb_h[:])
        eighth = persist.tile([CK, 128], F32R, tag="eighth")     # b_sq/8 lhsT
        nc.sync.dma_start(out=eighth[:], in_=ce_h[:])

        ms_sb = persist.tile([128, TT], F32, tag="ms")
        nc.sync.dma_start(out=ms_sb[:], in_=ms_h[:])
        p_sb = persist.tile([1, NCORE], F32, tag="p")
        nc.sync.dma_start(out=p_sb[:], in_=p_h[:])
        lv0 = persist.tile([128, NCORE], F32, tag="lv0")
        nc.sync.dma_start(out=lv0[:], in_=lv_h[0:128, :])
        lv1 = persist.tile([128, NCORE], F32, tag="lv1")
        nc.sync.dma_start(out=lv1[:], in_=lv_h[128:256, :])

        q_sb = persist.tile([128, NCORE], F32R, tag="q")
        # bsq2[h] holds b_sq/8 for half h duplicated twice along free dim.
        bsq2 = [persist.tile([128, 2 * NH], F32, tag=f"bsq2_{h}", name=f"bsq2_{h}")
                for h in (0, 1)]

        with tc.tile_pool(name="setup", bufs=1) as setup:
            qk_sb = setup.tile([CK, NCORE], F32, tag="qk")
            nc.sync.dma_start(out=qk_sb[:], in_=qk_h[:])
            qe_sb = setup.tile([CK, NCORE], F32, tag="qe")
            nc.sync.dma_start(out=qe_sb[:], in_=qe_h[:])
            t1 = setup.tile([CK, NCORE], F32, tag="t1")
            t2 = setup.tile([CK, NCORE], F32R, tag="t2")

            # copy-then-mul keeps each DVE op to a single cross-engine wait
            nc.vector.tensor_copy(t1[:], qk_sb[:])
            nc.vector.tensor_mul(t1[:], t1[:], qe_sb[:])               # qe*qk
            nc.vector.tensor_scalar_mul(q_sb[0:CK, :], qe_sb[:], -0.125)
            nc.vector.tensor_scalar_mul(q_sb[CK:128, :], t1[:], 0.25)
            nc.vector.tensor_mul(t2[:], t1[:], qk_sb[:])               # qe*qk^2

            pb = ps_pair.tile([128, 1024], F32, tag="pair")
            nc.tensor.matmul(pb[:, 0:NH], eighth[:], t2[:, 0:NH],
                             start=True, stop=True)
            nc.tensor.matmul(pb[:, 512:512 + NH], eighth[:],
                             t2[:, NH:2 * NH], start=True, stop=True)
            for h in (0, 1):
                src = pb[:, 0:NH] if h == 0 else pb[:, 512:512 + NH]
                nc.vector.tensor_copy(bsq2[h][:, 0:NH], src)
                nc.vector.tensor_copy(bsq2[h][:, NH:2 * NH], src)

        # ---- resident mvT ------------------------------------------------
        mv_sb = mvpool.tile([128, TT * CV], F32R, tag="mvres")
        for g in range(9):
            src = mv_h[g * 2048:(g + 1) * 2048, :].rearrange("(j p) c -> p j c", p=128)
            dst = mv_sb[:, g * 16 * CV:(g + 1) * 16 * CV].rearrange(
                "p (j c) -> p j c", c=CV)
            nc.sync.dma_start(out=dst, in_=src)

        # ---- main passes -------------------------------------------------
        for h in (0, 1):
            qh = q_sb[:, h * NH:(h + 1) * NH]
            r_acc = [ps_acc.tile([128, NH], F32, tag=f"r{k}", name=f"r{k}")
                     for k in (0, 1)]
            z_acc = ps_acc.tile([2, NH], F32, tag="z")

            e_tiles = {}
            pairs = {}
            m2c = None
            for pr in range(NPAIR + SKEW):
                if pr < NPAIR:
                    ta, tb = 2 * pr, 2 * pr + 1
                    if ta % MKCH == 0:
                        m2c = m2pool.tile([128, 128 * MKCH], F32R, tag="m2c")
                        nc.sync.dma_start(
                            out=m2c[CK:128, :],
                            in_=mk_h[:, ta * 128:(ta + MKCH) * 128])
                        nc.gpsimd.tensor_mul(m2c[0:CK, :], m2c[CK:128, :].bitcast(F32),
                                             m2c[CK:128, :].bitcast(F32))
                    pair = ps_pair.tile([128, 1024], F32, tag="pair")
                    pairs[pr] = pair
                    for i, t in enumerate((ta, tb)):
                        nc.tensor.matmul(
                            pair[:, 512 * i:512 * i + NH],
                            m2c[:, (t % MKCH) * 128:(t % MKCH + 1) * 128],
                            qh, start=True, stop=True)
                    # psum -= b_sq/8 (both halves in one strided op)
                    pview = pair[:].rearrange("p (k x) -> p k x", x=512)[:, :, 0:NH]
                    bview = bsq2[h][:].rearrange("p (k x) -> p k x", x=NH)
                    nc.vector.tensor_sub(pview, pview, bview)
                    for i, t in enumerate((ta, tb)):
                        e = epool.tile([128, NH], F32R, tag="E")
                        nc.scalar.activation(e[:], pair[:, 512 * i:512 * i + NH],
                                             EXP, scale=ms_sb[:, t:t + 1])
                        e_tiles[t] = e
                if pr >= SKEW:
                    for t in (2 * (pr - SKEW), 2 * (pr - SKEW) + 1):
                        e = e_tiles.pop(t)
                        st, sp = (t == 0), (t == TT - 1)
                        er = e[:]
                        for k in (0, 1):
                            nc.tensor.matmul(
                                r_acc[k][:],
                                mv_sb[:, t * CV + k * 128:t * CV + (k + 1) * 128],
                                er, start=st, stop=sp)
                        nc.tensor.matmul(z_acc[:], ones_z[:], er,
                                         start=st, stop=sp)

            # ---- finalize pass h ----------------------------------------
            ph = p_sb[:, h * NH:(h + 1) * NH]
            rz = smpool.tile([1, NH], F32, tag="rz")
            nc.vector.reciprocal(rz[:], z_acc[0:1, :])
            w1 = smpool.tile([1, NH], F32R, tag="w1")
            nc.vector.tensor_mul(w1[:], rz[:], ph)          # p / Z
            w2 = smpool.tile([1, NH], F32R, tag="w2")
            nc.vector.tensor_scalar_mul(w2[:], ph, -1.0)
            nc.vector.tensor_scalar_add(w2[:], w2[:], 1.0)  # 1 - p

            wps = ps_pair.tile([128, 1024], F32, tag="pair")
            nc.tensor.matmul(wps[:, 0:NH], ones_b[:], w1[:],
                             start=True, stop=True)
            nc.tensor.matmul(wps[:, 512:512 + NH], ones_b[:], w2[:],
                             start=True, stop=True)
            w1s = smpool.tile([128, NH], F32, tag="w1s")
            nc.vector.tensor_copy(w1s[:], wps[:, 0:NH])
            w2s = smpool.tile([128, NH], F32, tag="w2s")
            nc.vector.tensor_copy(w2s[:], wps[:, 512:512 + NH])

            for k, lvt in ((0, lv0), (1, lv1)):
                o = opool.tile([128, NH], F32, tag="O")
                tmp = opool.tile([128, NH], F32, tag="tmp")
                nc.vector.tensor_mul(o[:], r_acc[k][:], w1s[:])
                nc.vector.tensor_mul(tmp[:], lvt[:, h * NH:(h + 1) * NH], w2s[:])
                nc.vector.tensor_add(o[:], o[:], tmp[:])
                nc.sync.dma_start(
                    out=out_h[k * 128:(k + 1) * 128, h * NH:(h + 1) * NH],
                    in_=o[:])

    nc.finalize()
    return nc


def _get_program():
    if "nc" not in _CACHE:
        _CACHE["nc"] = build_program()
    return _CACHE["nc"]


def _make_in_maps(query_key, query_selection, memory_key, memory_shrinkage,
                  msk_value, uncert_prob):
    qk = np.asarray(query_key, np.float32).reshape(B, CK, HW)
    qe = np.asarray(query_selection, np.float32).reshape(B, CK, HW)
    mk = np.asarray(memory_key, np.float32).reshape(B, CK, THW)
    ms = np.asarray(memory_shrinkage, np.float32).reshape(B, THW)
    mv = np.asarray(msk_value, np.float32).reshape(B, CV, THW)
    lv = np.asarray(msk_value, np.float32).reshape(B, CV, T, HW)[:, :, T - 1, :]
    p = np.asarray(uncert_prob, np.float32).reshape(B, HW)

    in_maps = []
    for core in range(8):
        b, s = divmod(core, 4)
        sl = slice(s * NCORE, (s + 1) * NCORE)
        in_maps.append({
            "c_onesz": np.ones((128, 2), np.float32),
            "c_onesb": np.ones((1, 128), np.float32),
            "c_eighth": np.full((CK, 128), 0.125, np.float32),
            "qk": np.ascontiguousarray(qk[b, :, sl]),
            "qe": np.ascontiguousarray(qe[b, :, sl]),
            "mk": np.ascontiguousarray(mk[b]),
            "msT": np.ascontiguousarray(ms[b].reshape(TT, 128).T),
            "mvT": np.ascontiguousarray(mv[b].T),
            "lv": np.ascontiguousarray(lv[b, :, sl]),
            "p": np.ascontiguousarray(p[b, sl]).reshape(1, NCORE),
        })
    return in_maps


def kernel(**inputs):
    nc = _get_program()
    in_maps = _make_in_maps(**inputs)
    res = run_bass_kernel_spmd(nc, in_maps, list(range(8)))
    out = np.empty((B, 1, CV, HW), np.float32)
    for core in range(8):
        b, s = divmod(core, 4)
        out[b, 0, :, s * NCORE:(s + 1) * NCORE] = res.results[core]["out"]
    return out.reshape(B, 1, CV, H, W)


if __name__ == "__main__":
    rng = np.random.default_rng(0)
    dummy = {
        "query_key": rng.standard_normal((B, CK, H, W), np.float32),
        "query_selection": rng.random((B, CK, H, W), np.float32),
        "memory_key": rng.standard_normal((B, CK, T, H, W), np.float32),
        "memory_shrinkage": rng.random((B, 1, T, H, W), np.float32),
        "msk_value": rng.standard_normal((B, 1, CV, T, H, W), np.float32),
        "uncert_prob": rng.random((B, 1, H, W), np.float32),
    }
    out = kernel(**dummy)
    print("out", out.shape, out.dtype, float(np.abs(out).mean()))
